# revision 14
# baseline (speedup 1.0000x reference)
"""Trainium2 Bass kernel for a 1-D correlation volume (stereo cost volume).

Problem: out[n, i, h, w] = (1/C) * sum_c x1[n,c,h,w] * x2[n,c,h,w-i],
zero where w-i < 0, for i in 0..D (D=64).
Shapes: x1, x2 = [8, 128, 128, 256] f32; out = [8, 65, 128, 256] f32.

Sharding: data-parallel over the batch dim — each of the 8 NeuronCores
processes one sample end to end (no collectives).

Per-core algorithm (fully on-chip, no DRAM scratch)
---------------------------------------------------
For each w-tile (two tiles of 128), the contraction over c is a banded
Gram matmul per row h:
    band[p, col] = (1/C) * sum_c x1[c, ts+p] * x2[c, (ts-64)+col]
holding out[i, h, ts+p] at col = p + 64 - i.  Extraction of the 65
diagonals needs per-partition offsets while p is the partition dim, so:
transpose each band column slice [p, h] -> [h, p] on the TensorEngine.
With h on partitions, band column c maps to the output staging tile at
free offset i*128 + p = (64-c)*128 + p*129 -- one strided copy per
column, PSUM -> osb directly; the drain IS the diagonal extraction.

Engine-cost notes: ACT costs (N+352)/1.2 ns per op, so stage-1 psum
drains are batched 3 h-rows per ACTIVATE via bank-aligned psum slots;
the band is staged bf16 (rel err ~3e-3, gate is 2e-2); per-column
drains alternate between DVE and ACT.
"""

import numpy as np

import concourse.bass as bass
import concourse.tile as tile
from concourse import bacc, masks, mybir
from concourse.bass_utils import run_bass_kernel_spmd

# Problem constants (hardcoded per the harness contract).
B = 8          # batch == number of cores
C = 128        # channels (matmul K)
H = 128        # rows
W = 256        # cols
D = 64         # max disparity
ND = D + 1     # number of disparities (65)
T = 128        # w-tile size (matmul M)
NT = W // T    # 2 w-tiles
BANDC = T + D  # 192 band columns per w-tile
BW = T + BANDC  # 320 band columns total in the merged band tile
HB = 8         # h rows per input streaming block
HQ = 3         # h rows per stage-1 ACT drain (psum slots of 512 f32)
CG = 8         # band cols per PSUM bank in stage 2

F32 = mybir.dt.float32
BF16 = mybir.dt.bfloat16

NCORES = B


def _corr_body(tc, out_d, x1_d, x2_d):
    nc = tc.nc
    with (
        tc.tile_pool(name="io", bufs=2) as io_pool,
        tc.tile_pool(name="band", bufs=1) as band_pool,
        tc.tile_pool(name="osb", bufs=1) as osb_pool,
        tc.tile_pool(name="single", bufs=1) as single_pool,
        tc.tile_pool(name="mm_psum", bufs=2, space="PSUM") as mm_psum,
        tc.tile_pool(name="tp_psum", bufs=2, space="PSUM") as tp_psum,
    ):
        ident = single_pool.tile([T, T], BF16, tag="ident", name="ident")
        masks.make_identity(nc, ident[:])

        # Merged band tile [p, h, col']: col' 0:128 = w-tile-0 band cols
        # 64:192; col' 128:320 = w-tile-1 band cols 0:192.
        bband = band_pool.tile([T, H, BW], BF16, tag="bband", name="bband")

        # Output staging per w-tile: osb[h, i, p].  For w-tile 0 the cells
        # with p < i stay zero (w-i < 0), so that buffer is pre-zeroed.
        osb0 = osb_pool.tile([H, ND, T], F32, tag="osb0", name="osb0")
        osb1 = osb_pool.tile([H, ND, T], F32, tag="osb1", name="osb1")
        nc.gpsimd.memset(osb0[:], 0.0)

        # ---- Stage 1: banded Gram matmuls, band staged as [p, h, col'] ----
        for hb in range(0, H, HB):
            x1t = io_pool.tile([C, HB, W], F32, tag="x1t", name="x1t")
            nc.sync.dma_start(x1t[:], x1_d[:, hb : hb + HB, :])
            x2t = io_pool.tile([C, HB, W], F32, tag="x2t", name="x2t")
            nc.sync.dma_start(x2t[:], x2_d[:, hb : hb + HB, :])

            for h0 in range(0, HB, HQ):
                hn = min(HQ, HB - h0)
                # psum slots are 512 f32 (bank-aligned) per h row.
                pt = mm_psum.tile([T, HQ, 512], F32, tag="pt", name="pt")
                for hl in range(hn):
                    hh = h0 + hl
                    nc.tensor.matmul(
                        pt[:, hl, 0:T],
                        x1t[:, hh, 0:T],
                        x2t[:, hh, 0:T],
                        start=True,
                        stop=True,
                    )
                    nc.tensor.matmul(
                        pt[:, hl, T:BW],
                        x1t[:, hh, T : 2 * T],
                        x2t[:, hh, T - D : W],
                        start=True,
                        stop=True,
                    )
                h = hb + h0
                src = bass.AP(
                    pt.tensor, pt.offset, [[HQ * 512, T], [512, hn], [1, BW]]
                )
                nc.scalar.mul(bband[:, h : h + hn, :], src, 1.0 / C)

        # ---- Stage 2 per w-tile: transpose cols, drain = extraction ----
        # Band col c (0..191) holds out[i, h, w0 + p] with p = c - 64 + i,
        # valid i in [max(0, 64-c), min(64, 191-c)].  After the transpose
        # the drain writes osb at offset i*T + p = (64-c)*T + p*(T+1).
        drains = [
            lambda d, s: nc.vector.tensor_copy(d, s),
            lambda d, s: nc.scalar.copy(d, s),
        ]
        ei = 0
        for t in range(NT):
            osb = osb0 if t == 0 else osb1
            col0 = D if t == 0 else 0  # first valid band col
            cbase = 0 if t == 0 else T  # col' offset in bband
            for cg in range(col0, BANDC, CG):
                ptr = tp_psum.tile([H, CG, T], BF16, tag="ptr", name="ptr")
                for cl in range(CG):
                    col = cg + cl
                    nc.tensor.transpose(
                        ptr[:, cl, :],
                        bband[:, :, cbase + col - col0],
                        ident[:],
                    )
                if D <= cg and cg + CG - 1 < T:
                    # Middle group: all 8 cols have the full 65 i's.
                    # src addr = cl*T + p = cl*(T+1) + (cg-D) + i
                    # dst addr = i*T + p = i*(T+1) + (cg-D) + cl
                    src = bass.AP(
                        ptr.tensor,
                        ptr.offset + (cg - D),
                        [[CG * T, H], [T + 1, CG], [1, ND]],
                    )
                    dst = bass.AP(
                        osb.tensor,
                        osb.offset + (cg - D),
                        [[ND * T, H], [1, CG], [T + 1, ND]],
                    )
                    drains[ei % 2](dst, src)
                    ei += 1
                else:
                    for cl in range(CG):
                        c = cg + cl
                        i_lo = max(0, D - c)
                        i_hi = min(D, T + D - 1 - c)
                        ln = i_hi - i_lo + 1
                        p_lo = c - D + i_lo
                        src = bass.AP(
                            ptr.tensor,
                            ptr.offset + cl * T + p_lo,
                            [[CG * T, H], [1, ln]],
                        )
                        dst = bass.AP(
                            osb.tensor,
                            osb.offset + i_lo * T + p_lo,
                            [[ND * T, H], [T + 1, ln]],
                        )
                        drains[ei % 2](dst, src)
                        ei += 1

            dst = bass.AP(out_d, t * T, [[W, H], [H * W, ND], [1, T]])
            nc.sync.dma_start(dst, osb[:])


_NC_CACHE = None


def _build_nc():
    global _NC_CACHE
    if _NC_CACHE is not None:
        return _NC_CACHE
    nc = bacc.Bacc("TRN2")
    x1_d = nc.declare_dram_parameter("x1", [C, H, W], F32, isOutput=False)
    x2_d = nc.declare_dram_parameter("x2", [C, H, W], F32, isOutput=False)
    out_d = nc.declare_dram_parameter("out", [ND, H, W], F32, isOutput=True)
    with tile.TileContext(nc) as tc:
        _corr_body(tc, out_d, x1_d, x2_d)
    nc.finalize()
    _NC_CACHE = nc
    return nc


def _make_in_maps(x1, x2):
    return [
        {
            "x1": np.ascontiguousarray(x1[n], dtype=np.float32),
            "x2": np.ascontiguousarray(x2[n], dtype=np.float32),
        }
        for n in range(B)
    ]


def kernel(x1: np.ndarray, x2: np.ndarray) -> np.ndarray:
    assert x1.shape == (B, C, H, W) and x2.shape == (B, C, H, W)
    nc = _build_nc()
    res = run_bass_kernel_spmd(nc, _make_in_maps(x1, x2), list(range(B)))
    return np.stack([res.results[n]["out"] for n in range(B)], axis=0)


# revision 20
# speedup vs baseline: 1.0215x; 1.0215x over previous
"""Trainium2 Bass kernel for a 1-D correlation volume (stereo cost volume).

Problem: out[n, i, h, w] = (1/C) * sum_c x1[n,c,h,w] * x2[n,c,h,w-i],
zero where w-i < 0, for i in 0..D (D=64).
Shapes: x1, x2 = [8, 128, 128, 256] f32; out = [8, 65, 128, 256] f32.

Sharding: data-parallel over the batch dim — each of the 8 NeuronCores
processes one sample end to end (no collectives).

Per-core algorithm (fully on-chip, no DRAM scratch)
---------------------------------------------------
For each w-tile (two tiles of 128), the contraction over c is a banded
Gram matmul per row h:
    band[p, col] = (1/C) * sum_c x1[c, ts+p] * x2[c, (ts-64)+col]
holding out[i, h, ts+p] at col = p + 64 - i.  Extraction of the 65
diagonals needs per-partition offsets while p is the partition dim, so:
transpose each band column slice [p, h] -> [h, p] on the TensorEngine.
With h on partitions, band column c maps to the output staging tile at
free offset i*128 + p = (64-c)*128 + p*129 -- one strided copy per
column, PSUM -> osb directly; the drain IS the diagonal extraction.

Engine-cost notes: ACT costs (N+352)/1.2 ns per op, so stage-1 psum
drains are batched 3 h-rows per ACTIVATE via bank-aligned psum slots;
the band is staged bf16 (rel err ~3e-3, gate is 2e-2); per-column
drains alternate between DVE and ACT.
"""

import numpy as np

import concourse.bass as bass
import concourse.tile as tile
from concourse import bacc, masks, mybir
from concourse.bass_utils import run_bass_kernel_spmd

# Problem constants (hardcoded per the harness contract).
B = 8          # batch == number of cores
C = 128        # channels (matmul K)
H = 128        # rows
W = 256        # cols
D = 64         # max disparity
ND = D + 1     # number of disparities (65)
T = 128        # w-tile size (matmul M)
NT = W // T    # 2 w-tiles
BANDC = T + D  # 192 band columns per w-tile
BW = T + BANDC  # 320 band columns total in the merged band tile
HB = 8         # h rows per input streaming block
HQ = 3         # h rows per stage-1 ACT drain (psum slots of 512 f32)
CG = 8         # band cols per PSUM bank in stage 2

F32 = mybir.dt.float32
BF16 = mybir.dt.bfloat16

NCORES = B


def _corr_body(tc, out_d, x1_d, x2_d):
    nc = tc.nc
    with (
        tc.tile_pool(name="io", bufs=2) as io_pool,
        tc.tile_pool(name="band", bufs=1) as band_pool,
        tc.tile_pool(name="osb", bufs=1) as osb_pool,
        tc.tile_pool(name="single", bufs=1) as single_pool,
        tc.tile_pool(name="mm_psum", bufs=2, space="PSUM") as mm_psum,
        tc.tile_pool(name="tp_psum", bufs=2, space="PSUM") as tp_psum,
    ):
        ident = single_pool.tile([T, T], BF16, tag="ident", name="ident")
        masks.make_identity(nc, ident[:])

        # HAM warm-up: ~3.5us of back-to-back matmuls flips the PE clock
        # gate to 8/8 (2.4 GHz) before the real work starts.
        warm = single_pool.tile([T, D], BF16, tag="warm", name="warm")
        nc.gpsimd.memset(warm[:], 0.0)
        wps = mm_psum.tile([T, HQ, 512], F32, tag="pt", name="wps")
        for wi in range(14):
            nc.tensor.matmul(
                wps[0:D, 0, 0:D], warm[:], warm[:], start=True, stop=True
            )

        # Merged band tile [p, h, col']: col' 0:128 = w-tile-0 band cols
        # 64:192; col' 128:320 = w-tile-1 band cols 0:192.
        bband = band_pool.tile([T, H, BW], BF16, tag="bband", name="bband")

        # Output staging per w-tile: osb[h, i, p].  For w-tile 0 the cells
        # with p < i stay zero (w-i < 0), so that buffer is pre-zeroed.
        osb0 = osb_pool.tile([H, ND, T], F32, tag="osb0", name="osb0")
        osb1 = osb_pool.tile([H, ND, T], F32, tag="osb1", name="osb1")
        nc.gpsimd.memset(osb0[:], 0.0)

        # ---- Stage 1: banded Gram matmuls, band staged as [p, h, col'] ----
        for hb in range(0, H, HB):
            x1t = io_pool.tile([C, HB, W], F32, tag="x1t", name="x1t")
            nc.sync.dma_start(x1t[:], x1_d[:, hb : hb + HB, :])
            x2t = io_pool.tile([C, HB, W], F32, tag="x2t", name="x2t")
            nc.scalar.dma_start(x2t[:], x2_d[:, hb : hb + HB, :])

            for h0 in range(0, HB, HQ):
                hn = min(HQ, HB - h0)
                # psum slots are 512 f32 (bank-aligned) per h row.
                pt = mm_psum.tile([T, HQ, 512], F32, tag="pt", name="pt")
                for hl in range(hn):
                    hh = h0 + hl
                    nc.tensor.matmul(
                        pt[:, hl, 0:T],
                        x1t[:, hh, 0:T],
                        x2t[:, hh, 0:T],
                        start=True,
                        stop=True,
                    )
                    nc.tensor.matmul(
                        pt[:, hl, T:BW],
                        x1t[:, hh, T : 2 * T],
                        x2t[:, hh, T - D : W],
                        start=True,
                        stop=True,
                    )
                h = hb + h0
                src = bass.AP(
                    pt.tensor, pt.offset, [[HQ * 512, T], [512, hn], [1, BW]]
                )
                nc.scalar.mul(bband[:, h : h + hn, :], src, 1.0 / C)

        # ---- Stage 2 per w-tile: transpose cols, drain = extraction ----
        # Band col c (0..191) holds out[i, h, w0 + p] with p = c - 64 + i,
        # valid i in [max(0, 64-c), min(64, 191-c)].  After the transpose
        # the drain writes osb at offset i*T + p = (64-c)*T + p*(T+1).
        drains = [
            lambda d, s: nc.vector.tensor_copy(d, s),
            lambda d, s: nc.scalar.copy(d, s),
        ]
        ei = 0
        for t in range(NT):
            osb = osb0 if t == 0 else osb1
            col0 = D if t == 0 else 0  # first valid band col
            cbase = 0 if t == 0 else T  # col' offset in bband
            for cg in range(col0, BANDC, CG):
                # Transpose only the valid p-window of each column: the
                # moving operand is the identity column slice p_lo:p_lo+ln,
                # so psum slot cl holds out[...] indexed by j = p - p_lo.
                ptr = tp_psum.tile([H, CG, ND + 1], BF16, tag="ptr", name="ptr")
                lns = []
                for cl in range(CG):
                    c = cg + cl
                    p_lo = max(0, c - D)
                    ln = min(T - 1, c) - p_lo + 1
                    lns.append((c, p_lo, ln))
                    nc.tensor.transpose(
                        ptr[:, cl, 0:ln],
                        bband[:, :, cbase + c - col0],
                        ident[:, p_lo : p_lo + ln],
                    )
                if D <= cg and cg + CG - 1 < T:
                    # Middle group: every col has the full 65 i's, j = i.
                    # src addr = cl*ND + i; dst = i*(T+1) + (cg-D) + cl
                    src = bass.AP(
                        ptr.tensor,
                        ptr.offset,
                        [[CG * (ND + 1), H], [ND + 1, CG], [1, ND]],
                    )
                    dst = bass.AP(
                        osb.tensor,
                        osb.offset + (cg - D),
                        [[ND * T, H], [1, CG], [T + 1, ND]],
                    )
                    drains[ei % 2](dst, src)
                    ei += 1
                else:
                    for cl in range(CG):
                        c, p_lo, ln = lns[cl]
                        i_lo = p_lo + D - c
                        src = bass.AP(
                            ptr.tensor,
                            ptr.offset + cl * (ND + 1),
                            [[CG * (ND + 1), H], [1, ln]],
                        )
                        dst = bass.AP(
                            osb.tensor,
                            osb.offset + i_lo * T + p_lo,
                            [[ND * T, H], [T + 1, ln]],
                        )
                        drains[ei % 2](dst, src)
                        ei += 1

            dst = bass.AP(out_d, t * T, [[W, H], [H * W, ND], [1, T]])
            nc.sync.dma_start(dst, osb[:])


_NC_CACHE = None


def _build_nc():
    global _NC_CACHE
    if _NC_CACHE is not None:
        return _NC_CACHE
    nc = bacc.Bacc("TRN2")
    x1_d = nc.declare_dram_parameter("x1", [C, H, W], F32, isOutput=False)
    x2_d = nc.declare_dram_parameter("x2", [C, H, W], F32, isOutput=False)
    out_d = nc.declare_dram_parameter("out", [ND, H, W], F32, isOutput=True)
    with tile.TileContext(nc) as tc:
        _corr_body(tc, out_d, x1_d, x2_d)
    nc.finalize()
    _NC_CACHE = nc
    return nc


def _make_in_maps(x1, x2):
    return [
        {
            "x1": np.ascontiguousarray(x1[n], dtype=np.float32),
            "x2": np.ascontiguousarray(x2[n], dtype=np.float32),
        }
        for n in range(B)
    ]


def kernel(x1: np.ndarray, x2: np.ndarray) -> np.ndarray:
    assert x1.shape == (B, C, H, W) and x2.shape == (B, C, H, W)
    nc = _build_nc()
    res = run_bass_kernel_spmd(nc, _make_in_maps(x1, x2), list(range(B)))
    return np.stack([res.results[n]["out"] for n in range(B)], axis=0)
